# revision 16
# baseline (speedup 1.0000x reference)
"""DeepSeekV3 MLA attention on 8 Trainium2 NeuronCores.

Sharding: DP2 (batch) x TP4 (heads). Core c handles batch c//4 and heads
[4*(c%4), 4*(c%4)+4). Each core computes a partial output (its heads' slice
of the row-parallel wo matmul); the host sums the 4 partials per batch.

All on-device tensors use transposed [dim, seq] layouts so every matmul
contracts along the partition axis. The wq columns are reordered host-side so
the RoPE lo/hi halves of all 4 heads land as partition-aligned [128, s] tiles.
"""

import math
import numpy as np
import ml_dtypes

import concourse.bass as bass
import concourse.mybir as mybir
import concourse.tile as tile
from concourse import bacc
from concourse.bass_utils import run_bass_kernel_spmd

B, S, E, H = 2, 2048, 2048, 16
NOPE, ROPE, VD = 128, 64, 128
QH = NOPE + ROPE  # 192
LORA = 512
ROPE_THETA = 10000.0
EPS = 1e-6

N_CORES = 8
HPC = H // 4          # heads per core = 4
SM_SCALE = QH ** -0.5

F32 = mybir.dt.float32
F32R = mybir.dt.float32r
BF16 = mybir.dt.bfloat16

# dtype knobs
DT_PROJ = BF16   # q/kva projection inputs (xT, wq, wkva)
DT_SC = BF16     # scores inputs (qT, kT)
DT_KVB = BF16    # kvb matmul inputs (ckv_nT, wkvb)
DT_V = BF16      # AV rhs (V) / probs lhsT
DT_WO = BF16     # wo matmul inputs (attnT, wo)

NP_OF = {BF16: ml_dtypes.bfloat16, F32: np.float32, F32R: np.float32}

SCH = 512          # seq chunk
NSC = S // SCH     # 4
NSB = S // 128     # 16
NE = E // 128      # 16
NC_ = LORA // 128  # 4


def build_nc():
    nc = bacc.Bacc("TRN2", target_bir_lowering=False, debug=False)

    xt = nc.dram_tensor("xt", [E, S], DT_PROJ, kind="ExternalInput")
    wq = nc.dram_tensor("wq", [E, 6 * 128], DT_PROJ, kind="ExternalInput")
    wkva = nc.dram_tensor("wkva", [E, LORA + ROPE], DT_PROJ, kind="ExternalInput")
    wkvbk = nc.dram_tensor("wkvbk", [LORA, HPC * NOPE], DT_KVB, kind="ExternalInput")
    wkvbv = nc.dram_tensor("wkvbv", [LORA, HPC * VD], DT_KVB, kind="ExternalInput")
    wo = nc.dram_tensor("wo", [HPC * VD, E], DT_WO, kind="ExternalInput")
    kvsc = nc.dram_tensor("kvsc", [128, NC_], F32, kind="ExternalInput")
    cos4 = nc.dram_tensor("cos4", [128, S], BF16, kind="ExternalInput")
    sin4 = nc.dram_tensor("sin4", [128, S], BF16, kind="ExternalInput")
    trilm = nc.dram_tensor("trilm", [128, 128], BF16, kind="ExternalInput")
    ident = nc.dram_tensor("ident", [128, 128], BF16, kind="ExternalInput")
    out = nc.dram_tensor("out", [S, E], F32, kind="ExternalOutput")

    with tile.TileContext(nc) as tc:
        with (
            tc.tile_pool(name="persist", bufs=1) as pp,
            tc.tile_pool(name="tables", bufs=1) as tbl,
        ):
            # small constant tables (P3 users)
            tril_t = tbl.tile([128, 128], BF16)
            id_t = tbl.tile([128, 128], BF16)
            nc.sync.dma_start(tril_t[:], trilm[:])
            nc.sync.dma_start(id_t[:], ident[:])

            # activations that live across phases
            q_nope = [pp.tile([128, S], DT_SC, name=f"q_nope{h}") for h in range(HPC)]
            q_rope = [pp.tile([64, S], DT_SC, name=f"q_rope{h}") for h in range(HPC)]
            k_nope = [pp.tile([128, S], DT_SC, name=f"k_nope{h}") for h in range(HPC)]
            k_rope = pp.tile([64, S], DT_SC)
            ckv_raw = [pp.tile([128, S], BF16, name=f"ckv_raw{c}") for c in range(NC_)]

            # ---------- Phase 1: q + kva projections (stream x over e) ----------
            with (
                tc.tile_pool(name="p1w", bufs=1) as p1w,
                tc.tile_pool(name="p1rope", bufs=1) as p1r,
                tc.tile_pool(name="p1x", bufs=4) as p1x,
                tc.tile_pool(name="p1ps", bufs=8, space="PSUM") as p1ps,
                tc.tile_pool(name="p1tmp", bufs=2) as p1tmp,
            ):
                cos_t = p1w.tile([128, S], BF16)
                sin_t = p1w.tile([128, S], BF16)
                wq_t = p1w.tile([128, NE, 6 * 128], DT_PROJ)
                wkva_t = p1w.tile([128, NE, LORA + ROPE], DT_PROJ)
                for e in range(NE):
                    nc.sync.dma_start(wq_t[:, e, :],
                                      wq[e * 128:(e + 1) * 128, :])
                    nc.sync.dma_start(wkva_t[:, e, :],
                                      wkva[e * 128:(e + 1) * 128, :])

                nc.sync.dma_start(cos_t[:], cos4[:])
                nc.sync.dma_start(sin_t[:], sin4[:])
                qlo4 = p1r.tile([128, S], BF16)
                qhi4 = p1r.tile([128, S], BF16)
                kpe_raw = p1r.tile([64, S], BF16)
                klo = p1r.tile([32, S], BF16)
                khi = p1r.tile([32, S], BF16)
                klo_r = p1r.tile([32, S], BF16)
                khi_r = p1r.tile([32, S], BF16)

                for sc in range(NSC):
                    ssl = bass.ts(sc, SCH)
                    # pass A: the 6 q-projection groups, e-outer so the PE
                    # streams right behind the x DMAs
                    psA = [p1ps.tile([128, SCH], F32, tag="p1ps",
                                     name=f"psA{sc}_{d}") for d in range(6)]
                    for e in range(NE):
                        xa = p1x.tile([128, SCH], DT_PROJ, name=f"xa{sc}_{e}",
                                      tag="xa")
                        nc.sync.dma_start(xa[:], xt[e * 128:(e + 1) * 128, ssl])
                        for d in range(6):
                            nc.tensor.matmul(
                                psA[d][:], wq_t[:, e, d * 128:(d + 1) * 128],
                                xa[:], start=(e == 0), stop=(e == NE - 1))
                    for h in range(HPC):
                        nc.scalar.copy(q_nope[h][:, ssl], psA[h][:])
                    ps_lo, ps_hi = psA[4], psA[5]
                    # rotate: lo' = lo*cos - hi*sin ; hi' = hi*cos + lo*sin
                    t1 = p1tmp.tile([128, SCH], BF16, tag="t1")
                    t2 = p1tmp.tile([128, SCH], BF16, tag="t2")
                    t3 = p1tmp.tile([128, SCH], BF16, tag="t3")
                    t4 = p1tmp.tile([128, SCH], BF16, tag="t4")
                    nc.vector.tensor_tensor(t1[:], ps_lo[:], cos_t[:, ssl],
                                            mybir.AluOpType.mult)
                    nc.vector.tensor_tensor(t2[:], ps_hi[:], sin_t[:, ssl],
                                            mybir.AluOpType.mult)
                    nc.vector.tensor_tensor(qlo4[:, ssl], t1[:], t2[:],
                                            mybir.AluOpType.subtract)
                    nc.vector.tensor_tensor(t3[:], ps_hi[:], cos_t[:, ssl],
                                            mybir.AluOpType.mult)
                    nc.vector.tensor_tensor(t4[:], ps_lo[:], sin_t[:, ssl],
                                            mybir.AluOpType.mult)
                    nc.vector.tensor_tensor(qhi4[:, ssl], t3[:], t4[:],
                                            mybir.AluOpType.add)
                    # pass B: ckv (4 groups) + k_pe, e-outer with re-DMA'd x
                    psB = [p1ps.tile([128, SCH], F32, tag="p1ps",
                                     name=f"psB{sc}_{c}") for c in range(NC_)]
                    psK = p1ps.tile([64, SCH], F32, tag="p1ps",
                                    name=f"psK{sc}")
                    for e in range(NE):
                        xb = p1x.tile([128, SCH], DT_PROJ, name=f"xb{sc}_{e}",
                                      tag="xb")
                        nc.sync.dma_start(xb[:], xt[e * 128:(e + 1) * 128, ssl])
                        for c in range(NC_):
                            nc.tensor.matmul(
                                psB[c][:], wkva_t[:, e, c * 128:(c + 1) * 128],
                                xb[:], start=(e == 0), stop=(e == NE - 1))
                        nc.tensor.matmul(psK[:], wkva_t[:, e, LORA:LORA + ROPE],
                                         xb[:], start=(e == 0),
                                         stop=(e == NE - 1))
                    for c in range(NC_):
                        nc.scalar.copy(ckv_raw[c][:, ssl], psB[c][:])
                    nc.scalar.copy(kpe_raw[:, ssl], psK[:])

                # split lo/hi to partition-0-based tiles (SBUF->SBUF DMA)
                nc.sync.dma_start(klo[:], kpe_raw[0:32, :])
                nc.sync.dma_start(khi[:], kpe_raw[32:64, :])
                # k_pe rope (partitions 0:32)
                kt1 = p1tmp.tile([32, S], BF16, tag="kt1", bufs=1)
                kt2 = p1tmp.tile([32, S], BF16, tag="kt2", bufs=1)
                nc.vector.tensor_tensor(kt1[:], klo[:], cos_t[0:32, :],
                                        mybir.AluOpType.mult)
                nc.vector.tensor_tensor(kt2[:], khi[:], sin_t[0:32, :],
                                        mybir.AluOpType.mult)
                nc.vector.tensor_tensor(klo_r[:], kt1[:], kt2[:],
                                        mybir.AluOpType.subtract)
                nc.vector.tensor_tensor(kt1[:], khi[:], cos_t[0:32, :],
                                        mybir.AluOpType.mult)
                nc.vector.tensor_tensor(kt2[:], klo[:], sin_t[0:32, :],
                                        mybir.AluOpType.mult)
                nc.vector.tensor_tensor(khi_r[:], kt1[:], kt2[:],
                                        mybir.AluOpType.add)
                # reassemble k_rope [64, S] and per-head q_rope [64, S]
                nc.sync.dma_start(k_rope[0:32, :], klo_r[:])
                nc.sync.dma_start(k_rope[32:64, :], khi_r[:])
                for h in range(HPC):
                    hs = bass.ts(h, 32)
                    nc.sync.dma_start(q_rope[h][0:32, :], qlo4[hs, :])
                    nc.sync.dma_start(q_rope[h][32:64, :], qhi4[hs, :])

            with tc.tile_pool(name="acts", bufs=1) as ap_:
                v_aug = [ap_.tile([128, NSB, VD + 1], DT_V, name=f"v_aug{h}")
                         for h in range(HPC)]
                attn_n = [ap_.tile([128, NSB, VD], BF16, name=f"attn_n{h}")
                          for h in range(HPC)]
                attn_t = [ap_.tile([128, S], DT_WO, name=f"attn_t{h}")
                          for h in range(HPC)]
                for h in range(HPC):
                    nc.vector.memset(v_aug[h][:, :, VD], 1.0)

                # ---------- Phase 2: RMSNorm + kvb ----------
                with (
                    tc.tile_pool(name="p2w", bufs=1) as p2w,
                    tc.tile_pool(name="p2ps", bufs=4, space="PSUM") as p2ps,
                    tc.tile_pool(name="p2tmp", bufs=1) as p2tmp,
                ):
                    kvsc_t = p2w.tile([128, NC_], F32)
                    ones_t = p2w.tile([128, 1], DT_KVB)
                    eps_t = p2w.tile([1, 1], F32)
                    nc.sync.dma_start(kvsc_t[:], kvsc[:])
                    nc.vector.memset(ones_t[:], 1.0)
                    nc.vector.memset(eps_t[:], EPS)
                    wbk_t = p2w.tile([128, NC_, HPC * NOPE], DT_KVB)
                    wbv_t = p2w.tile([128, NC_, HPC * VD], DT_KVB)
                    nc.sync.dma_start(
                        wbk_t[:], wkvbk.rearrange("(nc p) d -> p nc d", p=128))
                    nc.sync.dma_start(
                        wbv_t[:], wkvbv.rearrange("(nc p) d -> p nc d", p=128))
                    ckv_n = [p2tmp.tile([128, S], DT_KVB, name=f"ckv_n{c}")
                             for c in range(NC_)]
                    # pipeline per s-chunk: norm chain (DVE/ACT/GpSimd) of
                    # chunk sc overlaps kvb matmuls (PE) of chunk sc-1
                    for sc in range(NSC):
                        ssl = bass.ts(sc, SCH)
                        sq = [p2tmp.tile([128, SCH], BF16, name=f"sq{sc}_{c}",
                                         tag=f"sq{c}", bufs=2)
                              for c in range(NC_)]
                        for c in range(NC_):
                            nc.vector.tensor_tensor(
                                sq[c][:], ckv_raw[c][:, ssl],
                                ckv_raw[c][:, ssl], mybir.AluOpType.mult)
                        ps = p2ps.tile([1, SCH], F32, tag="ssq", bufs=2)
                        for c in range(NC_):
                            nc.tensor.matmul(ps[:], ones_t[:], sq[c][:],
                                             start=(c == 0), stop=(c == NC_ - 1))
                        s_row = p2tmp.tile([1, SCH], F32, tag="s_row", bufs=2)
                        r_row = p2tmp.tile([1, SCH], F32, tag="r_row", bufs=2)
                        r_bc = p2tmp.tile([128, SCH], F32, tag="r_bc", bufs=2)
                        nc.scalar.activation(s_row[:], ps[:],
                                             mybir.ActivationFunctionType.Sqrt,
                                             bias=eps_t[:], scale=1.0 / LORA)
                        nc.vector.reciprocal(r_row[:], s_row[:])
                        nc.gpsimd.partition_broadcast(r_bc[:], r_row[:])
                        # normalize: ckv_n = ckv_raw * kvsc[c] * r
                        for c in range(NC_):
                            nc.vector.scalar_tensor_tensor(
                                ckv_n[c][:, ssl], ckv_raw[c][:, ssl],
                                kvsc_t[:, c:c + 1], r_bc[:],
                                op0=mybir.AluOpType.mult,
                                op1=mybir.AluOpType.mult)
                        # k_nope[h] for this chunk
                        for h in range(HPC):
                            ps2 = p2ps.tile([128, SCH], F32, tag="p2ps")
                            for c in range(NC_):
                                nc.tensor.matmul(
                                    ps2[:], wbk_t[:, c, h * 128:(h + 1) * 128],
                                    ckv_n[c][:, ssl], start=(c == 0),
                                    stop=(c == NC_ - 1))
                            nc.scalar.copy(k_nope[h][:, ssl], ps2[:])
                        # V natural for this chunk's 4 s-blocks
                        for sbl in range(4):
                            sb = 4 * sc + sbl
                            psv = p2ps.tile([128, HPC * VD], F32, tag="p2psv",
                                            bufs=2)
                            for c in range(NC_):
                                nc.tensor.matmul(
                                    psv[:], ckv_n[c][:, bass.ts(sb, 128)],
                                    wbv_t[:, c, :], start=(c == 0),
                                    stop=(c == NC_ - 1))
                            for h in range(HPC):
                                if h % 2 == 0:
                                    nc.scalar.copy(v_aug[h][:, sb, 0:VD],
                                                   psv[:, h * VD:(h + 1) * VD])
                                else:
                                    nc.vector.tensor_copy(
                                        v_aug[h][:, sb, 0:VD],
                                        psv[:, h * VD:(h + 1) * VD])

                # ---------- Phase 3: attention ----------
                with (
                    tc.tile_pool(name="p3probs", bufs=2) as p3p,
                    tc.tile_pool(name="p3ps", bufs=3, space="PSUM") as p3ps,
                    tc.tile_pool(name="p3av", bufs=3, space="PSUM") as p3av,
                    tc.tile_pool(name="p3tmp", bufs=4) as p3tmp,
                    tc.tile_pool(name="p3tps", bufs=2, space="PSUM") as p3tps,
                ):
                    for h in range(HPC):
                        for qc in range(NSC):
                            qsl = bass.ts(qc, SCH)
                            nki = 4 * qc + 4  # k-blocks 0..nki-1
                            probs = p3p.tile([128, NSB, SCH], DT_V, tag="probs",
                                             name=f"probs_h{h}_q{qc}")
                            for ki in range(nki):
                                ksl = bass.ts(ki, 128)
                                js = max(0, ki - 4 * qc)  # first valid q-block
                                w = SCH - js * 128
                                vq = bass.ds(qc * SCH + js * 128, w)
                                vl = bass.ds(js * 128, w)
                                ps = p3ps.tile([128, SCH], F32, tag="sc")
                                nc.tensor.matmul(ps[:, vl], k_nope[h][:, ksl],
                                                 q_nope[h][:, vq], start=True,
                                                 stop=False)
                                nc.tensor.matmul(ps[:, vl], k_rope[:, ksl],
                                                 q_rope[h][:, vq], start=False,
                                                 stop=True)
                                nc.scalar.activation(
                                    probs[:, ki, vl], ps[:, vl],
                                    mybir.ActivationFunctionType.Exp,
                                    scale=SM_SCALE)
                            # causal mask on the 4 diagonal sub-tiles
                            for j in range(4):
                                qi = 4 * qc + j
                                nc.vector.tensor_tensor(
                                    probs[:, qi, bass.ts(j, 128)],
                                    probs[:, qi, bass.ts(j, 128)], tril_t[:],
                                    mybir.AluOpType.mult)
                            # AV per q-block
                            for j in range(4):
                                qi = 4 * qc + j
                                pa = p3av.tile([128, VD + 1], F32, tag="av")
                                for ki in range(qi + 1):
                                    nc.tensor.matmul(
                                        pa[:], probs[:, ki, bass.ts(j, 128)],
                                        v_aug[h][:, ki, :], start=(ki == 0),
                                        stop=(ki == qi))
                                rec = p3tmp.tile([128, 1], F32, tag="rec")
                                nc.vector.reciprocal(rec[:], pa[:, VD:VD + 1])
                                nc.vector.tensor_scalar_mul(
                                    attn_n[h][:, qi, :], pa[:, 0:VD], rec[:])
                        # transpose attn [s, v] -> attn_t [v, s] on PE
                        for sb in range(NSB):
                            pt = p3tps.tile([128, 128], BF16, tag="tp")
                            nc.tensor.transpose(pt[:], attn_n[h][:, sb, :],
                                                id_t[:])
                            if sb % 2 == 0:
                                nc.scalar.copy(attn_t[h][:, bass.ts(sb, 128)],
                                               pt[:])
                            else:
                                nc.vector.tensor_copy(
                                    attn_t[h][:, bass.ts(sb, 128)], pt[:])

                # ---------- Phase 4: output projection ----------
                with (
                    tc.tile_pool(name="p4w", bufs=1) as p4w,
                    tc.tile_pool(name="p4ps", bufs=1, space="PSUM") as p4ps,
                    tc.tile_pool(name="p4o", bufs=3) as p4o,
                ):
                    wo_t = [p4w.tile([128, E], DT_WO, name=f"wo_t{h}")
                            for h in range(HPC)]
                    for h in range(HPC):
                        nc.sync.dma_start(wo_t[h][:],
                                          wo[h * 128:(h + 1) * 128, :])
                    for sb in range(NSB):
                        pss = [p4ps.tile([128, SCH], F32, tag=f"po{ec}",
                                         name=f"po{sb}_{ec}", bufs=2)
                               for ec in range(NSC)]
                        for h in range(HPC):
                            for ec in range(NSC):
                                nc.tensor.matmul(
                                    pss[ec][:], attn_t[h][:, bass.ts(sb, 128)],
                                    wo_t[h][:, bass.ts(ec, SCH)],
                                    start=(h == 0), stop=(h == HPC - 1))
                        ot = p4o.tile([128, E], F32, tag="ot")
                        for ec in range(NSC):
                            if ec % 2 == 0:
                                nc.scalar.copy(ot[:, bass.ts(ec, SCH)],
                                               pss[ec][:])
                            else:
                                nc.vector.tensor_copy(ot[:, bass.ts(ec, SCH)],
                                                      pss[ec][:])
                        nc.sync.dma_start(out[bass.ts(sb, 128), :], ot[:])

    nc.finalize()
    return nc


def _prep_inputs(x, wq, wkv_a, wkv_b, wo, kv_norm_scale):
    """Build the 8 per-core input dicts (numpy, host-side sharding)."""
    x = np.asarray(x, np.float32)
    wq = np.asarray(wq, np.float32)
    wkv_a = np.asarray(wkv_a, np.float32)
    wkv_b = np.asarray(wkv_b, np.float32)
    wo = np.asarray(wo, np.float32)
    kv_norm_scale = np.asarray(kv_norm_scale, np.float32)

    bf = ml_dtypes.bfloat16
    pos = np.arange(S, dtype=np.float32)
    inv = 1.0 / (ROPE_THETA ** (np.arange(0, ROPE, 2, dtype=np.float32) / ROPE))
    ang = pos[:, None] * inv  # [S, 32]
    cosT = np.cos(ang).T  # [32, S]
    sinT = np.sin(ang).T
    cos4 = np.tile(cosT, (4, 1)).astype(bf)
    sin4 = np.tile(sinT, (4, 1)).astype(bf)
    tril = (np.arange(128)[None, :] >= np.arange(128)[:, None]).astype(bf)
    ident = np.eye(128, dtype=bf)
    kvsc = kv_norm_scale.reshape(NC_, 128).T.copy()  # [128, NC_]

    wq_r = wq.reshape(E, H, QH)
    wkv_b_r = wkv_b.reshape(LORA, H, NOPE + VD)
    wo_r = wo.reshape(H, VD, E)

    in_maps = []
    for c in range(N_CORES):
        b, hg = c // 4, c % 4
        hs = [4 * hg + j for j in range(HPC)]
        xt = np.ascontiguousarray(x[b].T).astype(NP_OF[DT_PROJ])
        # wq cols: nope h0..h3 | lo4 | hi4
        wq_loc = np.concatenate(
            [wq_r[:, h, 0:NOPE] for h in hs]
            + [np.concatenate([wq_r[:, h, NOPE:NOPE + 32] for h in hs], axis=1)]
            + [np.concatenate([wq_r[:, h, NOPE + 32:QH] for h in hs], axis=1)],
            axis=1).astype(NP_OF[DT_PROJ])
        wkvbk = np.concatenate([wkv_b_r[:, h, 0:NOPE] for h in hs],
                               axis=1).astype(NP_OF[DT_KVB])
        wkvbv = np.concatenate([wkv_b_r[:, h, NOPE:] for h in hs],
                               axis=1).astype(NP_OF[DT_KVB])
        wo_loc = np.concatenate([wo_r[h] for h in hs],
                                axis=0).astype(NP_OF[DT_WO])
        in_maps.append({
            "xt": xt,
            "wq": wq_loc,
            "wkva": wkv_a.astype(NP_OF[DT_PROJ]),
            "wkvbk": wkvbk,
            "wkvbv": wkvbv,
            "wo": wo_loc,
            "kvsc": kvsc,
            "cos4": cos4,
            "sin4": sin4,
            "trilm": tril,
            "ident": ident,
        })
    return in_maps


_LAST_EXEC_NS = None


def kernel(x, wq, wkv_a, wkv_b, wo, kv_norm_scale, _trace=False):
    global _LAST_EXEC_NS
    nc = build_nc()
    in_maps = _prep_inputs(x, wq, wkv_a, wkv_b, wo, kv_norm_scale)
    res = run_bass_kernel_spmd(nc, in_maps, list(range(N_CORES)), trace=_trace)
    _LAST_EXEC_NS = res.exec_time_ns
    out = np.zeros((B, S, E), np.float32)
    for c in range(N_CORES):
        out[c // 4] += res.results[c]["out"]
    return out


# revision 20
# speedup vs baseline: 1.0740x; 1.0740x over previous
"""DeepSeekV3 MLA attention on 8 Trainium2 NeuronCores.

Sharding: DP2 (batch) x TP4 (heads). Core c handles batch c//4 and heads
[4*(c%4), 4*(c%4)+4). Each core computes a partial output (its heads' slice
of the row-parallel wo matmul); the host sums the 4 partials per batch.

All on-device tensors use transposed [dim, seq] layouts so every matmul
contracts along the partition axis. The wq columns are reordered host-side so
the RoPE lo/hi halves of all 4 heads land as partition-aligned [128, s] tiles.
"""

import math
import numpy as np
import ml_dtypes

import concourse.bass as bass
import concourse.mybir as mybir
import concourse.tile as tile
from concourse import bacc
from concourse.bass_utils import run_bass_kernel_spmd

B, S, E, H = 2, 2048, 2048, 16
NOPE, ROPE, VD = 128, 64, 128
QH = NOPE + ROPE  # 192
LORA = 512
ROPE_THETA = 10000.0
EPS = 1e-6

N_CORES = 8
HPC = H // 4          # heads per core = 4
SM_SCALE = QH ** -0.5

F32 = mybir.dt.float32
F32R = mybir.dt.float32r
BF16 = mybir.dt.bfloat16

# dtype knobs
DT_PROJ = BF16   # q/kva projection inputs (xT, wq, wkva)
DT_SC = BF16     # scores inputs (qT, kT)
DT_KVB = BF16    # kvb matmul inputs (ckv_nT, wkvb)
DT_V = BF16      # AV rhs (V) / probs lhsT
DT_WO = BF16     # wo matmul inputs (attnT, wo)

NP_OF = {BF16: ml_dtypes.bfloat16, F32: np.float32, F32R: np.float32}

SCH = 512          # seq chunk
NSC = S // SCH     # 4
NSB = S // 128     # 16
NE = E // 128      # 16
NC_ = LORA // 128  # 4


def build_nc():
    nc = bacc.Bacc("TRN2", target_bir_lowering=False, debug=False)

    xt = nc.dram_tensor("xt", [E, S], DT_PROJ, kind="ExternalInput")
    wq = nc.dram_tensor("wq", [E, 6 * 128], DT_PROJ, kind="ExternalInput")
    wkva = nc.dram_tensor("wkva", [E, LORA + ROPE], DT_PROJ, kind="ExternalInput")
    wkvbk = nc.dram_tensor("wkvbk", [LORA, HPC * NOPE], DT_KVB, kind="ExternalInput")
    wkvbv = nc.dram_tensor("wkvbv", [LORA, HPC * VD], DT_KVB, kind="ExternalInput")
    wo = nc.dram_tensor("wo", [HPC * VD, E], DT_WO, kind="ExternalInput")
    kvsc = nc.dram_tensor("kvsc", [128, NC_], F32, kind="ExternalInput")
    cos4 = nc.dram_tensor("cos4", [128, S], BF16, kind="ExternalInput")
    sin4 = nc.dram_tensor("sin4", [128, S], BF16, kind="ExternalInput")
    trilm = nc.dram_tensor("trilm", [128, 128], BF16, kind="ExternalInput")
    ident = nc.dram_tensor("ident", [128, 128], BF16, kind="ExternalInput")
    out = nc.dram_tensor("out", [S, E], F32, kind="ExternalOutput")

    with tile.TileContext(nc) as tc:
        with (
            tc.tile_pool(name="persist", bufs=1) as pp,
            tc.tile_pool(name="tables", bufs=1) as tbl,
        ):
            # small constant tables (P3 users)
            tril_t = tbl.tile([128, 128], BF16)
            id_t = tbl.tile([128, 128], BF16)
            nc.sync.dma_start(tril_t[:], trilm[:])
            nc.sync.dma_start(id_t[:], ident[:])

            # activations that live across phases
            q_nope = [pp.tile([128, S], DT_SC, name=f"q_nope{h}") for h in range(HPC)]
            q_rope = [pp.tile([64, S], DT_SC, name=f"q_rope{h}") for h in range(HPC)]
            k_nope = [pp.tile([128, S], DT_SC, name=f"k_nope{h}") for h in range(HPC)]
            k_rope = pp.tile([64, S], DT_SC)
            ckv_raw = [pp.tile([128, S], BF16, name=f"ckv_raw{c}") for c in range(NC_)]

            # ---------- Phase 1: q + kva projections (stream x over e) ----------
            with (
                tc.tile_pool(name="p1w", bufs=1) as p1w,
                tc.tile_pool(name="p1rope", bufs=1) as p1r,
                tc.tile_pool(name="p1x", bufs=20) as p1x,
                tc.tile_pool(name="p1ps", bufs=8, space="PSUM") as p1ps,
                tc.tile_pool(name="p1tmp", bufs=2) as p1tmp,
            ):
                cos_t = p1w.tile([128, S], BF16)
                sin_t = p1w.tile([128, S], BF16)
                wq_t = p1w.tile([128, NE, 6 * 128], DT_PROJ)
                wkva_t = p1w.tile([128, NE, LORA + ROPE], DT_PROJ)
                # interleave chunk-0 x tiles with the weight stream so the
                # first matmul group can start almost immediately
                xts0 = [p1x.tile([128, SCH], DT_PROJ, name=f"xts0_{e}",
                                 tag="xts") for e in range(NE)]
                for e in range(NE):
                    nc.sync.dma_start(xts0[e][:], xt[e * 128:(e + 1) * 128,
                                                     0:SCH])
                    nc.sync.dma_start(wq_t[:, e, :],
                                      wq[e * 128:(e + 1) * 128, :])
                    nc.sync.dma_start(wkva_t[:, e, :],
                                      wkva[e * 128:(e + 1) * 128, :])

                nc.sync.dma_start(cos_t[:], cos4[:])
                nc.sync.dma_start(sin_t[:], sin4[:])
                qlo4 = p1r.tile([128, S], BF16)
                qhi4 = p1r.tile([128, S], BF16)
                kpe_raw = p1r.tile([64, S], BF16)
                klo = p1r.tile([32, S], BF16)
                khi = p1r.tile([32, S], BF16)
                klo_r = p1r.tile([32, S], BF16)
                khi_r = p1r.tile([32, S], BF16)

                for sc in range(NSC):
                    ssl = bass.ts(sc, SCH)
                    if sc == 0:
                        xts = xts0
                    else:
                        xts = [p1x.tile([128, SCH], DT_PROJ,
                                        name=f"xts{sc}_{e}", tag="xts")
                               for e in range(NE)]
                        for e in range(NE):
                            nc.sync.dma_start(
                                xts[e][:], xt[e * 128:(e + 1) * 128, ssl])
                    # pass A: the 6 q-projection groups, e-outer so the PE
                    # streams right behind the x DMAs
                    psA = [p1ps.tile([128, SCH], F32, tag="p1ps",
                                     name=f"psA{sc}_{d}") for d in range(6)]
                    for e in range(NE):
                        for d in range(6):
                            nc.tensor.matmul(
                                psA[d][:], wq_t[:, e, d * 128:(d + 1) * 128],
                                xts[e][:], start=(e == 0), stop=(e == NE - 1))
                    for h in range(HPC):
                        nc.scalar.copy(q_nope[h][:, ssl], psA[h][:])
                    ps_lo, ps_hi = psA[4], psA[5]
                    # rotate: lo' = lo*cos - hi*sin ; hi' = hi*cos + lo*sin
                    t1 = p1tmp.tile([128, SCH], BF16, tag="t1")
                    t2 = p1tmp.tile([128, SCH], BF16, tag="t2")
                    t3 = p1tmp.tile([128, SCH], BF16, tag="t1", name=f"t3_{sc}")
                    t4 = p1tmp.tile([128, SCH], BF16, tag="t2", name=f"t4_{sc}")
                    nc.vector.tensor_tensor(t1[:], ps_lo[:], cos_t[:, ssl],
                                            mybir.AluOpType.mult)
                    nc.vector.tensor_tensor(t2[:], ps_hi[:], sin_t[:, ssl],
                                            mybir.AluOpType.mult)
                    nc.vector.tensor_tensor(qlo4[:, ssl], t1[:], t2[:],
                                            mybir.AluOpType.subtract)
                    nc.vector.tensor_tensor(t3[:], ps_hi[:], cos_t[:, ssl],
                                            mybir.AluOpType.mult)
                    nc.vector.tensor_tensor(t4[:], ps_lo[:], sin_t[:, ssl],
                                            mybir.AluOpType.mult)
                    nc.vector.tensor_tensor(qhi4[:, ssl], t3[:], t4[:],
                                            mybir.AluOpType.add)
                    # pass B: ckv (4 groups) + k_pe, e-outer with re-DMA'd x
                    psB = [p1ps.tile([128, SCH], F32, tag="p1ps",
                                     name=f"psB{sc}_{c}") for c in range(NC_)]
                    psK = p1ps.tile([64, SCH], F32, tag="p1ps",
                                    name=f"psK{sc}")
                    for e in range(NE):
                        for c in range(NC_):
                            nc.tensor.matmul(
                                psB[c][:], wkva_t[:, e, c * 128:(c + 1) * 128],
                                xts[e][:], start=(e == 0), stop=(e == NE - 1))
                        nc.tensor.matmul(psK[:], wkva_t[:, e, LORA:LORA + ROPE],
                                         xts[e][:], start=(e == 0),
                                         stop=(e == NE - 1))
                    for c in range(NC_):
                        nc.scalar.copy(ckv_raw[c][:, ssl], psB[c][:])
                    nc.scalar.copy(kpe_raw[:, ssl], psK[:])

                # split lo/hi to partition-0-based tiles (SBUF->SBUF DMA)
                nc.sync.dma_start(klo[:], kpe_raw[0:32, :])
                nc.sync.dma_start(khi[:], kpe_raw[32:64, :])
                # k_pe rope (partitions 0:32)
                kt1 = p1tmp.tile([32, S], BF16, tag="kt1", bufs=1)
                kt2 = p1tmp.tile([32, S], BF16, tag="kt2", bufs=1)
                nc.vector.tensor_tensor(kt1[:], klo[:], cos_t[0:32, :],
                                        mybir.AluOpType.mult)
                nc.vector.tensor_tensor(kt2[:], khi[:], sin_t[0:32, :],
                                        mybir.AluOpType.mult)
                nc.vector.tensor_tensor(klo_r[:], kt1[:], kt2[:],
                                        mybir.AluOpType.subtract)
                nc.vector.tensor_tensor(kt1[:], khi[:], cos_t[0:32, :],
                                        mybir.AluOpType.mult)
                nc.vector.tensor_tensor(kt2[:], klo[:], sin_t[0:32, :],
                                        mybir.AluOpType.mult)
                nc.vector.tensor_tensor(khi_r[:], kt1[:], kt2[:],
                                        mybir.AluOpType.add)
                # reassemble k_rope [64, S] and per-head q_rope [64, S]
                nc.sync.dma_start(k_rope[0:32, :], klo_r[:])
                nc.sync.dma_start(k_rope[32:64, :], khi_r[:])
                for h in range(HPC):
                    hs = bass.ts(h, 32)
                    nc.sync.dma_start(q_rope[h][0:32, :], qlo4[hs, :])
                    nc.sync.dma_start(q_rope[h][32:64, :], qhi4[hs, :])

            with tc.tile_pool(name="acts", bufs=1) as ap_:
                v_aug = [ap_.tile([128, NSB, VD + 1], DT_V, name=f"v_aug{h}")
                         for h in range(HPC)]
                attn_n = [ap_.tile([128, NSB, VD], BF16, name=f"attn_n{h}")
                          for h in range(HPC)]
                attn_t = [ap_.tile([128, S], DT_WO, name=f"attn_t{h}")
                          for h in range(HPC)]
                for h in range(HPC):
                    nc.vector.memset(v_aug[h][:, :, VD], 1.0)

                # ---------- Phase 2: RMSNorm + kvb ----------
                with (
                    tc.tile_pool(name="p2w", bufs=1) as p2w,
                    tc.tile_pool(name="p2ps", bufs=4, space="PSUM") as p2ps,
                    tc.tile_pool(name="p2tmp", bufs=1) as p2tmp,
                ):
                    kvsc_t = p2w.tile([128, NC_], F32)
                    ones_t = p2w.tile([128, 1], DT_KVB)
                    eps_t = p2w.tile([1, 1], F32)
                    nc.sync.dma_start(kvsc_t[:], kvsc[:])
                    nc.vector.memset(ones_t[:], 1.0)
                    nc.vector.memset(eps_t[:], EPS)
                    wbk_t = p2w.tile([128, NC_, HPC * NOPE], DT_KVB)
                    wbv_t = p2w.tile([128, NC_, HPC * VD], DT_KVB)
                    nc.sync.dma_start(
                        wbk_t[:], wkvbk.rearrange("(nc p) d -> p nc d", p=128))
                    nc.sync.dma_start(
                        wbv_t[:], wkvbv.rearrange("(nc p) d -> p nc d", p=128))
                    ckv_n = [p2tmp.tile([128, S], DT_KVB, name=f"ckv_n{c}")
                             for c in range(NC_)]
                    # pipeline per s-chunk: norm chain (DVE/ACT/GpSimd) of
                    # chunk sc overlaps kvb matmuls (PE) of chunk sc-1
                    for sc in range(NSC):
                        ssl = bass.ts(sc, SCH)
                        sq = [p2tmp.tile([128, SCH], BF16, name=f"sq{sc}_{c}",
                                         tag=f"sq{c}", bufs=2)
                              for c in range(NC_)]
                        for c in range(NC_):
                            nc.vector.tensor_tensor(
                                sq[c][:], ckv_raw[c][:, ssl],
                                ckv_raw[c][:, ssl], mybir.AluOpType.mult)
                        ps = p2ps.tile([1, SCH], F32, tag="ssq", bufs=2)
                        for c in range(NC_):
                            nc.tensor.matmul(ps[:], ones_t[:], sq[c][:],
                                             start=(c == 0), stop=(c == NC_ - 1))
                        s_row = p2tmp.tile([1, SCH], F32, tag="s_row", bufs=2)
                        r_row = p2tmp.tile([1, SCH], F32, tag="r_row", bufs=2)
                        r_bc = p2tmp.tile([128, SCH], F32, tag="r_bc", bufs=2)
                        nc.scalar.activation(s_row[:], ps[:],
                                             mybir.ActivationFunctionType.Sqrt,
                                             bias=eps_t[:], scale=1.0 / LORA)
                        nc.vector.reciprocal(r_row[:], s_row[:])
                        nc.gpsimd.partition_broadcast(r_bc[:], r_row[:])
                        # normalize: ckv_n = ckv_raw * kvsc[c] * r
                        for c in range(NC_):
                            nc.vector.scalar_tensor_tensor(
                                ckv_n[c][:, ssl], ckv_raw[c][:, ssl],
                                kvsc_t[:, c:c + 1], r_bc[:],
                                op0=mybir.AluOpType.mult,
                                op1=mybir.AluOpType.mult)
                        # k_nope[h] for this chunk
                        for h in range(HPC):
                            ps2 = p2ps.tile([128, SCH], F32, tag="p2ps")
                            for c in range(NC_):
                                nc.tensor.matmul(
                                    ps2[:], wbk_t[:, c, h * 128:(h + 1) * 128],
                                    ckv_n[c][:, ssl], start=(c == 0),
                                    stop=(c == NC_ - 1))
                            nc.scalar.copy(k_nope[h][:, ssl], ps2[:])
                        # V natural for this chunk's 4 s-blocks
                        for sbl in range(4):
                            sb = 4 * sc + sbl
                            psv = p2ps.tile([128, HPC * VD], F32, tag="p2psv",
                                            bufs=2)
                            for c in range(NC_):
                                nc.tensor.matmul(
                                    psv[:], ckv_n[c][:, bass.ts(sb, 128)],
                                    wbv_t[:, c, :], start=(c == 0),
                                    stop=(c == NC_ - 1))
                            for h in range(HPC):
                                if h % 2 == 0:
                                    nc.scalar.copy(v_aug[h][:, sb, 0:VD],
                                                   psv[:, h * VD:(h + 1) * VD])
                                else:
                                    nc.vector.tensor_copy(
                                        v_aug[h][:, sb, 0:VD],
                                        psv[:, h * VD:(h + 1) * VD])

                # ---------- Phase 3: attention ----------
                with (
                    tc.tile_pool(name="p3probs", bufs=2) as p3p,
                    tc.tile_pool(name="p3ps", bufs=3, space="PSUM") as p3ps,
                    tc.tile_pool(name="p3av", bufs=3, space="PSUM") as p3av,
                    tc.tile_pool(name="p3tmp", bufs=4) as p3tmp,
                    tc.tile_pool(name="p3tps", bufs=2, space="PSUM") as p3tps,
                ):
                    for h in range(HPC):
                        for qc in range(NSC):
                            qsl = bass.ts(qc, SCH)
                            nki = 4 * qc + 4  # k-blocks 0..nki-1
                            probs = p3p.tile([128, NSB, SCH], DT_V, tag="probs",
                                             name=f"probs_h{h}_q{qc}")
                            for ki in range(nki):
                                ksl = bass.ts(ki, 128)
                                js = max(0, ki - 4 * qc)  # first valid q-block
                                w = SCH - js * 128
                                vq = bass.ds(qc * SCH + js * 128, w)
                                vl = bass.ds(js * 128, w)
                                ps = p3ps.tile([128, SCH], F32, tag="sc")
                                nc.tensor.matmul(ps[:, vl], k_nope[h][:, ksl],
                                                 q_nope[h][:, vq], start=True,
                                                 stop=False)
                                nc.tensor.matmul(ps[:, vl], k_rope[:, ksl],
                                                 q_rope[h][:, vq], start=False,
                                                 stop=True)
                                nc.scalar.activation(
                                    probs[:, ki, vl], ps[:, vl],
                                    mybir.ActivationFunctionType.Exp,
                                    scale=SM_SCALE)
                            # causal mask on the 4 diagonal sub-tiles
                            for j in range(4):
                                qi = 4 * qc + j
                                nc.vector.tensor_tensor(
                                    probs[:, qi, bass.ts(j, 128)],
                                    probs[:, qi, bass.ts(j, 128)], tril_t[:],
                                    mybir.AluOpType.mult)
                            # AV per q-block
                            for j in range(4):
                                qi = 4 * qc + j
                                pa = p3av.tile([128, VD + 1], F32, tag="av")
                                for ki in range(qi + 1):
                                    nc.tensor.matmul(
                                        pa[:], probs[:, ki, bass.ts(j, 128)],
                                        v_aug[h][:, ki, :], start=(ki == 0),
                                        stop=(ki == qi))
                                rec = p3tmp.tile([128, 1], F32, tag="rec")
                                nc.vector.reciprocal(rec[:], pa[:, VD:VD + 1])
                                nc.vector.tensor_scalar_mul(
                                    attn_n[h][:, qi, :], pa[:, 0:VD], rec[:])
                        # transpose attn [s, v] -> attn_t [v, s] on PE
                        for sb in range(NSB):
                            pt = p3tps.tile([128, 128], BF16, tag="tp")
                            nc.tensor.transpose(pt[:], attn_n[h][:, sb, :],
                                                id_t[:])
                            if sb % 2 == 0:
                                nc.scalar.copy(attn_t[h][:, bass.ts(sb, 128)],
                                               pt[:])
                            else:
                                nc.vector.tensor_copy(
                                    attn_t[h][:, bass.ts(sb, 128)], pt[:])

                # ---------- Phase 4: output projection ----------
                with (
                    tc.tile_pool(name="p4w", bufs=1) as p4w,
                    tc.tile_pool(name="p4ps", bufs=1, space="PSUM") as p4ps,
                    tc.tile_pool(name="p4o", bufs=3) as p4o,
                ):
                    wo_t = [p4w.tile([128, E], DT_WO, name=f"wo_t{h}")
                            for h in range(HPC)]
                    for h in range(HPC):
                        nc.sync.dma_start(wo_t[h][:],
                                          wo[h * 128:(h + 1) * 128, :])
                    for sb in range(NSB):
                        pss = [p4ps.tile([128, SCH], F32, tag=f"po{ec}",
                                         name=f"po{sb}_{ec}", bufs=2)
                               for ec in range(NSC)]
                        for h in range(HPC):
                            for ec in range(NSC):
                                nc.tensor.matmul(
                                    pss[ec][:], attn_t[h][:, bass.ts(sb, 128)],
                                    wo_t[h][:, bass.ts(ec, SCH)],
                                    start=(h == 0), stop=(h == HPC - 1))
                        ot = p4o.tile([128, E], F32, tag="ot")
                        for ec in range(NSC):
                            if ec % 2 == 0:
                                nc.scalar.copy(ot[:, bass.ts(ec, SCH)],
                                               pss[ec][:])
                            else:
                                nc.vector.tensor_copy(ot[:, bass.ts(ec, SCH)],
                                                      pss[ec][:])
                        nc.sync.dma_start(out[bass.ts(sb, 128), :], ot[:])

    nc.finalize()
    return nc


def _prep_inputs(x, wq, wkv_a, wkv_b, wo, kv_norm_scale):
    """Build the 8 per-core input dicts (numpy, host-side sharding)."""
    x = np.asarray(x, np.float32)
    wq = np.asarray(wq, np.float32)
    wkv_a = np.asarray(wkv_a, np.float32)
    wkv_b = np.asarray(wkv_b, np.float32)
    wo = np.asarray(wo, np.float32)
    kv_norm_scale = np.asarray(kv_norm_scale, np.float32)

    bf = ml_dtypes.bfloat16
    pos = np.arange(S, dtype=np.float32)
    inv = 1.0 / (ROPE_THETA ** (np.arange(0, ROPE, 2, dtype=np.float32) / ROPE))
    ang = pos[:, None] * inv  # [S, 32]
    cosT = np.cos(ang).T  # [32, S]
    sinT = np.sin(ang).T
    cos4 = np.tile(cosT, (4, 1)).astype(bf)
    sin4 = np.tile(sinT, (4, 1)).astype(bf)
    tril = (np.arange(128)[None, :] >= np.arange(128)[:, None]).astype(bf)
    ident = np.eye(128, dtype=bf)
    kvsc = kv_norm_scale.reshape(NC_, 128).T.copy()  # [128, NC_]

    wq_r = wq.reshape(E, H, QH)
    wkv_b_r = wkv_b.reshape(LORA, H, NOPE + VD)
    wo_r = wo.reshape(H, VD, E)

    in_maps = []
    for c in range(N_CORES):
        b, hg = c // 4, c % 4
        hs = [4 * hg + j for j in range(HPC)]
        xt = np.ascontiguousarray(x[b].T).astype(NP_OF[DT_PROJ])
        # wq cols: nope h0..h3 | lo4 | hi4
        wq_loc = np.concatenate(
            [wq_r[:, h, 0:NOPE] for h in hs]
            + [np.concatenate([wq_r[:, h, NOPE:NOPE + 32] for h in hs], axis=1)]
            + [np.concatenate([wq_r[:, h, NOPE + 32:QH] for h in hs], axis=1)],
            axis=1).astype(NP_OF[DT_PROJ])
        wkvbk = np.concatenate([wkv_b_r[:, h, 0:NOPE] for h in hs],
                               axis=1).astype(NP_OF[DT_KVB])
        wkvbv = np.concatenate([wkv_b_r[:, h, NOPE:] for h in hs],
                               axis=1).astype(NP_OF[DT_KVB])
        wo_loc = np.concatenate([wo_r[h] for h in hs],
                                axis=0).astype(NP_OF[DT_WO])
        in_maps.append({
            "xt": xt,
            "wq": wq_loc,
            "wkva": wkv_a.astype(NP_OF[DT_PROJ]),
            "wkvbk": wkvbk,
            "wkvbv": wkvbv,
            "wo": wo_loc,
            "kvsc": kvsc,
            "cos4": cos4,
            "sin4": sin4,
            "trilm": tril,
            "ident": ident,
        })
    return in_maps


_LAST_EXEC_NS = None


def kernel(x, wq, wkv_a, wkv_b, wo, kv_norm_scale, _trace=False):
    global _LAST_EXEC_NS
    nc = build_nc()
    in_maps = _prep_inputs(x, wq, wkv_a, wkv_b, wo, kv_norm_scale)
    res = run_bass_kernel_spmd(nc, in_maps, list(range(N_CORES)), trace=_trace)
    _LAST_EXEC_NS = res.exec_time_ns
    out = np.zeros((B, S, E), np.float32)
    for c in range(N_CORES):
        out[c // 4] += res.results[c]["out"]
    return out


# revision 22
# speedup vs baseline: 1.0882x; 1.0132x over previous
"""DeepSeekV3 MLA attention on 8 Trainium2 NeuronCores.

Sharding: DP2 (batch) x TP4 (heads). Core c handles batch c//4 and heads
[4*(c%4), 4*(c%4)+4). Each core computes a partial output (its heads' slice
of the row-parallel wo matmul); the host sums the 4 partials per batch.

All on-device tensors use transposed [dim, seq] layouts so every matmul
contracts along the partition axis. The wq columns are reordered host-side so
the RoPE lo/hi halves of all 4 heads land as partition-aligned [128, s] tiles.
"""

import math
import numpy as np
import ml_dtypes

import concourse.bass as bass
import concourse.mybir as mybir
import concourse.tile as tile
from concourse import bacc
from concourse.bass_utils import run_bass_kernel_spmd

B, S, E, H = 2, 2048, 2048, 16
NOPE, ROPE, VD = 128, 64, 128
QH = NOPE + ROPE  # 192
LORA = 512
ROPE_THETA = 10000.0
EPS = 1e-6

N_CORES = 8
HPC = H // 4          # heads per core = 4
SM_SCALE = QH ** -0.5

F32 = mybir.dt.float32
F32R = mybir.dt.float32r
BF16 = mybir.dt.bfloat16

# dtype knobs
DT_PROJ = BF16   # q/kva projection inputs (xT, wq, wkva)
DT_SC = BF16     # scores inputs (qT, kT)
DT_KVB = BF16    # kvb matmul inputs (ckv_nT, wkvb)
DT_V = BF16      # AV rhs (V) / probs lhsT
DT_WO = BF16     # wo matmul inputs (attnT, wo)

NP_OF = {BF16: ml_dtypes.bfloat16, F32: np.float32, F32R: np.float32}

SCH = 512          # seq chunk
NSC = S // SCH     # 4
NSB = S // 128     # 16
NE = E // 128      # 16
NC_ = LORA // 128  # 4


def build_nc():
    nc = bacc.Bacc("TRN2", target_bir_lowering=False, debug=False)

    xt = nc.dram_tensor("xt", [E, S], DT_PROJ, kind="ExternalInput")
    wq = nc.dram_tensor("wq", [E, 6 * 128], DT_PROJ, kind="ExternalInput")
    wkva = nc.dram_tensor("wkva", [E, LORA + ROPE], DT_PROJ, kind="ExternalInput")
    wkvbk = nc.dram_tensor("wkvbk", [LORA, HPC * NOPE], DT_KVB, kind="ExternalInput")
    wkvbv = nc.dram_tensor("wkvbv", [LORA, HPC * VD], DT_KVB, kind="ExternalInput")
    wo = nc.dram_tensor("wo", [HPC * VD, E], DT_WO, kind="ExternalInput")
    kvsc = nc.dram_tensor("kvsc", [128, NC_], F32, kind="ExternalInput")
    cos4 = nc.dram_tensor("cos4", [128, S], BF16, kind="ExternalInput")
    sin4 = nc.dram_tensor("sin4", [128, S], BF16, kind="ExternalInput")
    trilm = nc.dram_tensor("trilm", [128, 128], BF16, kind="ExternalInput")
    ident = nc.dram_tensor("ident", [128, 128], BF16, kind="ExternalInput")
    out = nc.dram_tensor("out", [S, E], F32, kind="ExternalOutput")

    with tile.TileContext(nc) as tc:
        with (
            tc.tile_pool(name="persist", bufs=1) as pp,
            tc.tile_pool(name="tables", bufs=1) as tbl,
            tc.tile_pool(name="ps", bufs=8, space="PSUM") as psp,
        ):
            tril_t = tbl.tile([128, 128], BF16)
            id_t = tbl.tile([128, 128], BF16)
            nc.sync.dma_start(tril_t[:], trilm[:])
            nc.sync.dma_start(id_t[:], ident[:])

            # per-chunk activation tiles (precise deps -> cross-phase overlap)
            q_nope = [[pp.tile([128, SCH], DT_SC, name=f"q_nope{h}_{c}")
                       for c in range(NSC)] for h in range(HPC)]
            q_rope = [[pp.tile([64, SCH], DT_SC, name=f"q_rope{h}_{c}")
                       for c in range(NSC)] for h in range(HPC)]
            k_rope = [pp.tile([64, SCH], DT_SC, name=f"k_rope{c}")
                      for c in range(NSC)]
            ckv_raw = [[pp.tile([128, SCH], BF16, name=f"ckv_raw{c}_{s}")
                        for s in range(NSC)] for c in range(NC_)]

            # ---------- Phase 1: q + kva projections ----------
            with (
                tc.tile_pool(name="p1w", bufs=1) as p1w,
                tc.tile_pool(name="p1rope", bufs=2) as p1r,
                tc.tile_pool(name="p1x", bufs=20) as p1x,
                tc.tile_pool(name="p1tmp", bufs=2) as p1tmp,
            ):
                cos_t = p1w.tile([128, S], BF16)
                sin_t = p1w.tile([128, S], BF16)
                wq_t = p1w.tile([128, NE, 6 * 128], DT_PROJ)
                wkva_t = p1w.tile([128, NE, LORA + ROPE], DT_PROJ)
                # interleave chunk-0 x tiles with the weight stream so the
                # first matmul group can start almost immediately
                xts0 = [p1x.tile([128, SCH], DT_PROJ, name=f"xts0_{e}",
                                 tag="xts") for e in range(NE)]
                for e in range(NE):
                    nc.sync.dma_start(xts0[e][:], xt[e * 128:(e + 1) * 128,
                                                     0:SCH])
                    nc.sync.dma_start(wq_t[:, e, :],
                                      wq[e * 128:(e + 1) * 128, :])
                    nc.sync.dma_start(wkva_t[:, e, :],
                                      wkva[e * 128:(e + 1) * 128, :])
                nc.sync.dma_start(cos_t[:], cos4[:])
                nc.sync.dma_start(sin_t[:], sin4[:])

                for sc in range(NSC):
                    ssl = bass.ts(sc, SCH)
                    if sc == 0:
                        xts = xts0
                    else:
                        xts = [p1x.tile([128, SCH], DT_PROJ,
                                        name=f"xts{sc}_{e}", tag="xts")
                               for e in range(NE)]
                        for e in range(NE):
                            nc.sync.dma_start(
                                xts[e][:], xt[e * 128:(e + 1) * 128, ssl])
                    # pass A: 6 q-projection groups, e-outer
                    psA = [psp.tile([128, SCH], F32, tag="ps",
                                    name=f"psA{sc}_{d}") for d in range(6)]
                    for e in range(NE):
                        for d in range(6):
                            nc.tensor.matmul(
                                psA[d][:], wq_t[:, e, d * 128:(d + 1) * 128],
                                xts[e][:], start=(e == 0), stop=(e == NE - 1))
                    for h in range(HPC):
                        nc.scalar.copy(q_nope[h][sc][:], psA[h][:])
                    ps_lo, ps_hi = psA[4], psA[5]
                    # q rope: lo' = lo*cos - hi*sin ; hi' = hi*cos + lo*sin
                    t1 = p1tmp.tile([128, SCH], BF16, tag="t1")
                    t2 = p1tmp.tile([128, SCH], BF16, tag="t2")
                    t3 = p1tmp.tile([128, SCH], BF16, tag="t1",
                                    name=f"t3_{sc}")
                    t4 = p1tmp.tile([128, SCH], BF16, tag="t2",
                                    name=f"t4_{sc}")
                    qlo4 = p1r.tile([128, SCH], BF16, tag="qlo4",
                                    name=f"qlo4_{sc}")
                    qhi4 = p1r.tile([128, SCH], BF16, tag="qhi4",
                                    name=f"qhi4_{sc}")
                    nc.vector.tensor_tensor(t1[:], ps_lo[:], cos_t[:, ssl],
                                            mybir.AluOpType.mult)
                    nc.vector.tensor_tensor(t2[:], ps_hi[:], sin_t[:, ssl],
                                            mybir.AluOpType.mult)
                    nc.vector.tensor_tensor(qlo4[:], t1[:], t2[:],
                                            mybir.AluOpType.subtract)
                    nc.vector.tensor_tensor(t3[:], ps_hi[:], cos_t[:, ssl],
                                            mybir.AluOpType.mult)
                    nc.vector.tensor_tensor(t4[:], ps_lo[:], sin_t[:, ssl],
                                            mybir.AluOpType.mult)
                    nc.vector.tensor_tensor(qhi4[:], t3[:], t4[:],
                                            mybir.AluOpType.add)
                    for h in range(HPC):
                        hs = bass.ts(h, 32)
                        nc.sync.dma_start(q_rope[h][sc][0:32, :], qlo4[hs, :])
                        nc.sync.dma_start(q_rope[h][sc][32:64, :], qhi4[hs, :])
                    # pass B: ckv (4 groups) + k_pe, e-outer on resident x
                    psB = [psp.tile([128, SCH], F32, tag="ps",
                                    name=f"psB{sc}_{c}") for c in range(NC_)]
                    psK = psp.tile([64, SCH], F32, tag="ps", name=f"psK{sc}")
                    for e in range(NE):
                        for c in range(NC_):
                            nc.tensor.matmul(
                                psB[c][:], wkva_t[:, e, c * 128:(c + 1) * 128],
                                xts[e][:], start=(e == 0), stop=(e == NE - 1))
                        nc.tensor.matmul(psK[:], wkva_t[:, e, LORA:LORA + ROPE],
                                         xts[e][:], start=(e == 0),
                                         stop=(e == NE - 1))
                    for c in range(NC_):
                        nc.scalar.copy(ckv_raw[c][sc][:], psB[c][:])
                    # k_pe rope for this chunk
                    kpe_raw = p1r.tile([64, SCH], BF16, tag="kpe",
                                       name=f"kpe_{sc}")
                    nc.scalar.copy(kpe_raw[:], psK[:])
                    klo = p1r.tile([32, SCH], BF16, tag="klo",
                                   name=f"klo_{sc}")
                    khi = p1r.tile([32, SCH], BF16, tag="khi",
                                   name=f"khi_{sc}")
                    nc.sync.dma_start(klo[:], kpe_raw[0:32, :])
                    nc.sync.dma_start(khi[:], kpe_raw[32:64, :])
                    kt1 = p1tmp.tile([32, SCH], BF16, tag="kt1",
                                     name=f"kt1_{sc}")
                    kt2 = p1tmp.tile([32, SCH], BF16, tag="kt2",
                                     name=f"kt2_{sc}")
                    klo_r = p1r.tile([32, SCH], BF16, tag="klor",
                                     name=f"klor_{sc}")
                    khi_r = p1r.tile([32, SCH], BF16, tag="khir",
                                     name=f"khir_{sc}")
                    nc.vector.tensor_tensor(kt1[:], klo[:], cos_t[0:32, ssl],
                                            mybir.AluOpType.mult)
                    nc.vector.tensor_tensor(kt2[:], khi[:], sin_t[0:32, ssl],
                                            mybir.AluOpType.mult)
                    nc.vector.tensor_tensor(klo_r[:], kt1[:], kt2[:],
                                            mybir.AluOpType.subtract)
                    nc.vector.tensor_tensor(kt1[:], khi[:], cos_t[0:32, ssl],
                                            mybir.AluOpType.mult)
                    nc.vector.tensor_tensor(kt2[:], klo[:], sin_t[0:32, ssl],
                                            mybir.AluOpType.mult)
                    nc.vector.tensor_tensor(khi_r[:], kt1[:], kt2[:],
                                            mybir.AluOpType.add)
                    nc.sync.dma_start(k_rope[sc][0:32, :], klo_r[:])
                    nc.sync.dma_start(k_rope[sc][32:64, :], khi_r[:])

            # ---------- fused per-chunk pipeline: norm+kvb / attn / wo ------
            with (
                tc.tile_pool(name="acts", bufs=1) as ap_,
                tc.tile_pool(name="a2", bufs=2) as a2,
                tc.tile_pool(name="arec", bufs=6) as arec,
                tc.tile_pool(name="aout", bufs=4) as aout,
            ):
                kvsc_t = ap_.tile([128, NC_], F32)
                ones_t = ap_.tile([128, 1], DT_KVB)
                eps_t = ap_.tile([1, 1], F32)
                wbk_t = ap_.tile([128, NC_, HPC * NOPE], DT_KVB)
                wbv_t = ap_.tile([128, NC_, HPC * VD], DT_KVB)
                wo_t = [ap_.tile([128, E], DT_WO, name=f"wo_t{h}")
                        for h in range(HPC)]
                nc.sync.dma_start(
                    wbk_t[:], wkvbk.rearrange("(nc p) d -> p nc d", p=128))
                nc.sync.dma_start(
                    wbv_t[:], wkvbv.rearrange("(nc p) d -> p nc d", p=128))
                for h in range(HPC):
                    nc.sync.dma_start(wo_t[h][:], wo[h * 128:(h + 1) * 128, :])
                nc.sync.dma_start(kvsc_t[:], kvsc[:])
                nc.vector.memset(ones_t[:], 1.0)
                nc.vector.memset(eps_t[:], EPS)

                k_nope = [[ap_.tile([128, SCH], DT_SC, name=f"k_nope{h}_{c}")
                           for c in range(NSC)] for h in range(HPC)]
                ckv_n = [[ap_.tile([128, SCH], DT_KVB, name=f"ckv_n{c}_{s}")
                          for s in range(NSC)] for c in range(NC_)]
                v_aug = [[ap_.tile([128, 4, VD + 1], DT_V,
                                   name=f"v_aug{h}_{c}")
                          for c in range(NSC)] for h in range(HPC)]
                attn_n = [[ap_.tile([128, 4, VD], BF16, name=f"attn_n{h}_{c}")
                           for c in range(NSC)] for h in range(HPC)]
                for h in range(HPC):
                    for c in range(NSC):
                        nc.vector.memset(v_aug[h][c][:, :, VD], 1.0)

                for qc in range(NSC):
                    # ---- P2(qc): RMSNorm + kvb + V ----
                    sq = [a2.tile([128, SCH], BF16, name=f"sq{qc}_{c}",
                                  tag=f"sq{c}", bufs=1) for c in range(NC_)]
                    for c in range(NC_):
                        nc.vector.tensor_tensor(sq[c][:], ckv_raw[c][qc][:],
                                                ckv_raw[c][qc][:],
                                                mybir.AluOpType.mult)
                    pss = psp.tile([1, SCH], F32, tag="ps", name=f"ssq{qc}")
                    for c in range(NC_):
                        nc.tensor.matmul(pss[:], ones_t[:], sq[c][:],
                                         start=(c == 0), stop=(c == NC_ - 1))
                    s_row = a2.tile([1, SCH], F32, tag="s_row",
                                    name=f"s_row{qc}")
                    r_row = a2.tile([1, SCH], F32, tag="r_row",
                                    name=f"r_row{qc}")
                    r_bc = a2.tile([128, SCH], F32, tag="r_bc",
                                   name=f"r_bc{qc}")
                    nc.scalar.activation(s_row[:], pss[:],
                                         mybir.ActivationFunctionType.Sqrt,
                                         bias=eps_t[:], scale=1.0 / LORA)
                    nc.vector.reciprocal(r_row[:], s_row[:])
                    nc.gpsimd.partition_broadcast(r_bc[:], r_row[:])
                    for c in range(NC_):
                        nc.vector.scalar_tensor_tensor(
                            ckv_n[c][qc][:], ckv_raw[c][qc][:],
                            kvsc_t[:, c:c + 1], r_bc[:],
                            op0=mybir.AluOpType.mult, op1=mybir.AluOpType.mult)
                    for h in range(HPC):
                        ps2 = psp.tile([128, SCH], F32, tag="ps",
                                       name=f"kvbk{qc}_{h}")
                        for c in range(NC_):
                            nc.tensor.matmul(
                                ps2[:], wbk_t[:, c, h * 128:(h + 1) * 128],
                                ckv_n[c][qc][:], start=(c == 0),
                                stop=(c == NC_ - 1))
                        nc.scalar.copy(k_nope[h][qc][:], ps2[:])
                    for sbl in range(4):
                        psv = psp.tile([128, HPC * VD], F32, tag="ps",
                                       name=f"v{qc}_{sbl}")
                        for c in range(NC_):
                            nc.tensor.matmul(
                                psv[:], ckv_n[c][qc][:, bass.ts(sbl, 128)],
                                wbv_t[:, c, :], start=(c == 0),
                                stop=(c == NC_ - 1))
                        for h in range(HPC):
                            if h % 2 == 0:
                                nc.scalar.copy(v_aug[h][qc][:, sbl, 0:VD],
                                               psv[:, h * VD:(h + 1) * VD])
                            else:
                                nc.vector.tensor_copy(
                                    v_aug[h][qc][:, sbl, 0:VD],
                                    psv[:, h * VD:(h + 1) * VD])

                    # ---- attention(qc) for all heads ----
                    nki = 4 * qc + 4
                    for h in range(HPC):
                        probs = a2.tile([128, nki, SCH], DT_V, tag="probs",
                                        name=f"probs_h{h}_q{qc}",
                                        padded_shape=[128, NSB, SCH])
                        for ki in range(nki):
                            kc, ko = ki // 4, ki % 4
                            js = max(0, ki - 4 * qc)
                            w = SCH - js * 128
                            vl = bass.ds(js * 128, w)
                            ps = psp.tile([128, SCH], F32, tag="ps",
                                          name=f"sc{qc}_{h}_{ki}")
                            nc.tensor.matmul(
                                ps[:, vl], k_nope[h][kc][:, bass.ts(ko, 128)],
                                q_nope[h][qc][:, vl], start=True, stop=False)
                            nc.tensor.matmul(
                                ps[:, vl], k_rope[kc][:, bass.ts(ko, 128)],
                                q_rope[h][qc][:, vl], start=False, stop=True)
                            nc.scalar.activation(
                                probs[:, ki, vl], ps[:, vl],
                                mybir.ActivationFunctionType.Exp,
                                scale=SM_SCALE)
                        for j in range(4):
                            qi = 4 * qc + j
                            nc.vector.tensor_tensor(
                                probs[:, qi, bass.ts(j, 128)],
                                probs[:, qi, bass.ts(j, 128)], tril_t[:],
                                mybir.AluOpType.mult)
                        for j in range(4):
                            qi = 4 * qc + j
                            pa = psp.tile([128, VD + 1], F32, tag="ps",
                                          name=f"av{qc}_{h}_{j}")
                            for ki in range(qi + 1):
                                nc.tensor.matmul(
                                    pa[:], probs[:, ki, bass.ts(j, 128)],
                                    v_aug[h][ki // 4][:, ki % 4, :],
                                    start=(ki == 0), stop=(ki == qi))
                            rec = arec.tile([128, 1], F32, tag="rec",
                                            name=f"rec{qc}_{h}_{j}")
                            nc.vector.reciprocal(rec[:], pa[:, VD:VD + 1])
                            nc.vector.tensor_scalar_mul(
                                attn_n[h][qc][:, j, :], pa[:, 0:VD], rec[:])

                    # ---- transpose + wo for this chunk's 4 s-blocks ----
                    att = [a2.tile([128, SCH], DT_WO, tag=f"att{h}",
                                   name=f"att{h}_{qc}") for h in range(HPC)]
                    for h in range(HPC):
                        for sbl in range(4):
                            pt = psp.tile([128, 128], BF16, tag="ps",
                                          name=f"tp{qc}_{h}_{sbl}")
                            nc.tensor.transpose(pt[:], attn_n[h][qc][:, sbl, :],
                                                id_t[:])
                            if sbl % 2 == 0:
                                nc.scalar.copy(att[h][:, bass.ts(sbl, 128)],
                                               pt[:])
                            else:
                                nc.vector.tensor_copy(
                                    att[h][:, bass.ts(sbl, 128)], pt[:])
                    for sbl in range(4):
                        sb = 4 * qc + sbl
                        for ec in range(NSC):
                            ps_o = psp.tile([128, SCH], F32, tag="ps",
                                            name=f"po{sb}_{ec}")
                            for h in range(HPC):
                                nc.tensor.matmul(
                                    ps_o[:], att[h][:, bass.ts(sbl, 128)],
                                    wo_t[h][:, bass.ts(ec, SCH)],
                                    start=(h == 0), stop=(h == HPC - 1))
                            ot = aout.tile([128, SCH], F32, tag="ot",
                                           name=f"ot{sb}_{ec}")
                            if ec % 2 == 0:
                                nc.scalar.copy(ot[:], ps_o[:])
                            else:
                                nc.vector.tensor_copy(ot[:], ps_o[:])
                            nc.sync.dma_start(
                                out[bass.ts(sb, 128), bass.ts(ec, SCH)],
                                ot[:])

    nc.finalize()
    return nc


def _prep_inputs(x, wq, wkv_a, wkv_b, wo, kv_norm_scale):
    """Build the 8 per-core input dicts (numpy, host-side sharding)."""
    x = np.asarray(x, np.float32)
    wq = np.asarray(wq, np.float32)
    wkv_a = np.asarray(wkv_a, np.float32)
    wkv_b = np.asarray(wkv_b, np.float32)
    wo = np.asarray(wo, np.float32)
    kv_norm_scale = np.asarray(kv_norm_scale, np.float32)

    bf = ml_dtypes.bfloat16
    pos = np.arange(S, dtype=np.float32)
    inv = 1.0 / (ROPE_THETA ** (np.arange(0, ROPE, 2, dtype=np.float32) / ROPE))
    ang = pos[:, None] * inv  # [S, 32]
    cosT = np.cos(ang).T  # [32, S]
    sinT = np.sin(ang).T
    cos4 = np.tile(cosT, (4, 1)).astype(bf)
    sin4 = np.tile(sinT, (4, 1)).astype(bf)
    tril = (np.arange(128)[None, :] >= np.arange(128)[:, None]).astype(bf)
    ident = np.eye(128, dtype=bf)
    kvsc = kv_norm_scale.reshape(NC_, 128).T.copy()  # [128, NC_]

    wq_r = wq.reshape(E, H, QH)
    wkv_b_r = wkv_b.reshape(LORA, H, NOPE + VD)
    wo_r = wo.reshape(H, VD, E)

    in_maps = []
    for c in range(N_CORES):
        b, hg = c // 4, c % 4
        hs = [4 * hg + j for j in range(HPC)]
        xt = np.ascontiguousarray(x[b].T).astype(NP_OF[DT_PROJ])
        # wq cols: nope h0..h3 | lo4 | hi4
        wq_loc = np.concatenate(
            [wq_r[:, h, 0:NOPE] for h in hs]
            + [np.concatenate([wq_r[:, h, NOPE:NOPE + 32] for h in hs], axis=1)]
            + [np.concatenate([wq_r[:, h, NOPE + 32:QH] for h in hs], axis=1)],
            axis=1).astype(NP_OF[DT_PROJ])
        wkvbk = np.concatenate([wkv_b_r[:, h, 0:NOPE] for h in hs],
                               axis=1).astype(NP_OF[DT_KVB])
        wkvbv = np.concatenate([wkv_b_r[:, h, NOPE:] for h in hs],
                               axis=1).astype(NP_OF[DT_KVB])
        wo_loc = np.concatenate([wo_r[h] for h in hs],
                                axis=0).astype(NP_OF[DT_WO])
        in_maps.append({
            "xt": xt,
            "wq": wq_loc,
            "wkva": wkv_a.astype(NP_OF[DT_PROJ]),
            "wkvbk": wkvbk,
            "wkvbv": wkvbv,
            "wo": wo_loc,
            "kvsc": kvsc,
            "cos4": cos4,
            "sin4": sin4,
            "trilm": tril,
            "ident": ident,
        })
    return in_maps


_LAST_EXEC_NS = None


def kernel(x, wq, wkv_a, wkv_b, wo, kv_norm_scale, _trace=False):
    global _LAST_EXEC_NS
    nc = build_nc()
    in_maps = _prep_inputs(x, wq, wkv_a, wkv_b, wo, kv_norm_scale)
    res = run_bass_kernel_spmd(nc, in_maps, list(range(N_CORES)), trace=_trace)
    _LAST_EXEC_NS = res.exec_time_ns
    out = np.zeros((B, S, E), np.float32)
    for c in range(N_CORES):
        out[c // 4] += res.results[c]["out"]
    return out
